# revision 1
# baseline (speedup 1.0000x reference)
"""Trainium2 Bass kernel for nn_Attention (dense transformer attention w/ QK-LayerNorm).

Sharding: sequence-parallel over 8 cores. Core c handles batch b = c//2,
token half h = c%2 (512 tokens). K/V are computed redundantly for the full
batch element on both cores of a pair (cheaper than the measured ~130us
pairwise AllGather); Q only for local tokens. No collectives.

Dataflow is transpose-free: Q/K are produced channel-major ([C, tok]) so the
QK-LayerNorm reductions over C become ones-vector matmuls on the partition
axis, and S^T = K_h Q_h^T tiles feed softmax-exp directly; V is produced
token-major with an appended ones-column per head so the PV matmul emits the
softmax denominator as an extra output row. All matmuls run in float32r
(TF32-rate on the PE at N>=256, fp32 storage).

Engine partition-access rule honored throughout: writes and ACT reads start
at 32-aligned partitions; only DVE reads use unaligned bases.
"""

import numpy as np

B, N, C = 4, 1024, 1024
H, D = 16, 64
LN_EPS = 1e-5
N_CORES = 8
TL = 512          # local tokens per core
KT = 8            # channel tiles of 128
SCALE = D ** -0.5

_COMPILED = None


def _build():
    import concourse.bacc as bacc
    import concourse.tile as tile
    import concourse.mybir as mybir

    F32 = mybir.dt.float32
    F32R = mybir.dt.float32r
    AF = mybir.ActivationFunctionType
    OP = mybir.AluOpType

    nc = bacc.Bacc("TRN2", target_bir_lowering=False, debug=False,
                   num_devices=N_CORES)

    xT_d = nc.dram_tensor("xT", [C, N], F32R, kind="ExternalInput").ap()
    qkvwT_d = nc.dram_tensor("qkvwT", [C, 3 * C], F32R, kind="ExternalInput").ap()
    projwT_d = nc.dram_tensor("projwT", [C, C], F32R, kind="ExternalInput").ap()
    wsums_d = nc.dram_tensor("wsums", [128, 16], F32R, kind="ExternalInput").ap()
    params_d = nc.dram_tensor("params", [128, 56], F32, kind="ExternalInput").ap()
    bsum_d = nc.dram_tensor("bsumC", [2, 1], F32, kind="ExternalInput").ap()
    selc_d = nc.dram_tensor("selc", [2, 256], F32R, kind="ExternalInput").ap()
    out_d = nc.dram_tensor("out", [C, TL], F32, kind="ExternalOutput").ap()

    with tile.TileContext(nc) as tc:
        with tc.tile_pool(name="persist", bufs=1) as pers, \
             tc.tile_pool(name="sq", bufs=2) as sqp, \
             tc.tile_pool(name="small", bufs=1) as smallp, \
             tc.tile_pool(name="ps", bufs=8, space="PSUM") as ps:

            khat = pers.tile([128, KT * N], F32R, tag="khat")       # 4MB
            vful = pers.tile([128, 8 * 1040], F32R, tag="vful")     # 4.06MB
            qhat = pers.tile([128, KT * TL], F32R, tag="qhat")      # 2MB

            params = smallp.tile([128, 56], F32, tag="params")
            wsums = smallp.tile([128, 16], F32R, tag="wsums")
            bsum = smallp.tile([2, 1], F32, tag="bsum")
            sel = smallp.tile([128, 4], F32R, tag="sel")  # e0=[1,0], e1=[0,1]
            st = smallp.tile([2, 7 * 512], F32, tag="st")
            stmm = smallp.tile([2, 2 * 512], F32R, tag="stmm")
            selc = smallp.tile([2, 256], F32R, tag="selc")
            ones_big = smallp.tile([128, 128], F32R, tag="ones_big")

            def sl(i):
                return st[:, i * 512:(i + 1) * 512]

            def smm(i):
                return stmm[:, i * 512:(i + 1) * 512]

            def prm(grp, kt):  # qn_w qn_b kn_w kn_b qb kb projb
                return params[:, grp * 8 + kt: grp * 8 + kt + 1]

            nc.sync.dma_start(params[:], params_d[:])
            nc.sync.dma_start(wsums[:], wsums_d[:])
            nc.sync.dma_start(bsum[:], bsum_d[:])
            nc.sync.dma_start(selc[:], selc_d[:])
            nc.vector.memset(ones_big[:].bitcast(F32), 1.0)
            nc.vector.memset(sel[:].bitcast(F32), 0.0)
            nc.vector.memset(sel[:, 0:1].bitcast(F32), 1.0)   # e0 col0
            nc.vector.memset(sel[:, 3:4].bitcast(F32), 1.0)   # e1 col1

            with tc.tile_pool(name="xp", bufs=1) as xpool, \
                 tc.tile_pool(name="wpool", bufs=16) as wpool:
                xT = xpool.tile([128, KT * N], F32R, tag="xT")      # 4MB
                rstdb = xpool.tile([128, 2 * N + 2 * TL], F32, tag="rstdb")
                xsrc = xT_d.rearrange("(a p) t -> p a t", p=128)
                nc.sync.dma_start(xT[:].rearrange("p (a t) -> p a t", a=KT), xsrc)

                def wslabs(pfx, col0):
                    tiles = []
                    for g in range(2):
                        for i in range(KT):
                            t = wpool.tile([128, 512], F32R, tag="w",
                                           name=f"{pfx}{g}_{i}")
                            nc.sync.dma_start(
                                t[:], qkvwT_d[i * 128:(i + 1) * 128,
                                              col0 + g * 512: col0 + (g + 1) * 512])
                            tiles.append(t)
                    return tiles

                # ---- token sums of (q,k) ----
                sums_ps = [ps.tile([2, 512], F32, tag="ps", name=f"sums_ps{i}")
                           for i in range(2)]
                for kt in range(KT):
                    for nh in range(2):
                        nc.tensor.matmul(
                            sums_ps[nh][:], wsums[:, kt * 2: kt * 2 + 2],
                            xT[:, kt * N + nh * 512: kt * N + (nh + 1) * 512],
                            start=(kt == 0), stop=(kt == KT - 1))
                nc.vector.tensor_copy(sl(0), sums_ps[0][:])
                nc.vector.tensor_copy(sl(1), sums_ps[1][:])

                # ---- K phase (channel-major, all N tokens) ----
                wk = wslabs("wk", C)
                for grp in range(2):
                    for mi in range(4):
                        m = grp * 4 + mi
                        for nh in range(2):
                            acc = ps.tile([128, 512], F32, tag="ps")
                            for kt in range(KT):
                                nc.tensor.matmul(
                                    acc[:],
                                    wk[grp * KT + kt][:, mi * 128:(mi + 1) * 128],
                                    xT[:, kt * N + nh * 512: kt * N + (nh + 1) * 512],
                                    start=(kt == 0), stop=(kt == KT - 1))
                            dst = khat[:, m * N + nh * 512: m * N + (nh + 1) * 512]
                            nc.scalar.activation(dst, acc[:], AF.Identity,
                                                 bias=prm(5, m))

                # ---- Q phase (channel-major, local tokens) ----
                wq = wslabs("wq", 0)
                for grp in range(2):
                    for mi in range(4):
                        m = grp * 4 + mi
                        acc = ps.tile([128, 512], F32, tag="ps")
                        for kt in range(KT):
                            nc.tensor.matmul(
                                acc[:],
                                wq[grp * KT + kt][:, mi * 128:(mi + 1) * 128],
                                xT[:, kt * N: kt * N + TL],
                                start=(kt == 0), stop=(kt == KT - 1))
                        dst = qhat[:, m * TL:(m + 1) * TL]
                        nc.scalar.activation(dst, acc[:], AF.Identity,
                                             bias=prm(4, m))

                # ---- ssq matmuls (squares on ACT overlapped Q matmuls) ----
                ssq_ps = [ps.tile([2, 512], F32, tag="ps", name=f"ssq_ps{i}")
                          for i in range(2)]
                for m in range(8):
                    for nh in range(2):
                        dst = khat[:, m * N + nh * 512: m * N + (nh + 1) * 512]
                        ksq = sqp.tile([128, 512], F32R, tag="sq")
                        nc.scalar.activation(ksq[:], dst.bitcast(F32), AF.Square)
                        nc.tensor.matmul(ssq_ps[nh][:], sel[:, 2:4], ksq[:],
                                         start=(m == 0), stop=(m == 7),
                                         skip_group_check=True)
                ssqQ_ps = ps.tile([2, 512], F32, tag="ps", name="ssqQ_ps")
                for m in range(8):
                    dst = qhat[:, m * TL:(m + 1) * TL]
                    qsq = sqp.tile([128, 512], F32R, tag="sq")
                    nc.scalar.activation(qsq[:], dst.bitcast(F32), AF.Square)
                    nc.tensor.matmul(ssqQ_ps[:], sel[:, 0:2], qsq[:],
                                     start=(m == 0), stop=(m == 7),
                                     skip_group_check=True)

                # ---- stats chains + broadcasts ----
                def chain(sums_slot, ssq_slot):
                    nc.vector.tensor_scalar(sl(4), sums_slot, 1.0 / C,
                                            bsum[:], OP.mult, OP.add)
                    nc.vector.tensor_mul(sl(5), sl(4), sl(4))
                    nc.vector.tensor_scalar(sl(6), ssq_slot, 1.0 / C,
                                            LN_EPS, OP.mult, OP.add)
                    nc.vector.tensor_sub(sl(6), sl(6), sl(5))
                    nc.vector.tensor_scalar_max(sl(6), sl(6), 1e-20)
                    nc.scalar.activation(sl(6), sl(6), AF.Ln)
                    nc.scalar.activation(smm(0), sl(6), AF.Exp, scale=-0.5)
                    nc.vector.tensor_mul(smm(1), sl(4), smm(0).bitcast(F32))

                def bcast(slot, row, dst_col):
                    bc_ps = ps.tile([128, 512], F32, tag="ps")
                    nc.tensor.matmul(bc_ps[:],
                                     selc[:, row * 128:(row + 1) * 128],
                                     smm(slot), start=True, stop=True)
                    nc.vector.tensor_copy(rstdb[:, dst_col:dst_col + 512],
                                          bc_ps[:])

                nc.vector.tensor_copy(sl(2), ssq_ps[0][:])
                nc.vector.tensor_copy(sl(3), ssq_ps[1][:])
                chain(sl(0), sl(2))
                bcast(0, 1, 0)
                bcast(1, 1, N)
                chain(sl(1), sl(3))
                bcast(0, 1, 512)
                bcast(1, 1, N + 512)
                nc.vector.tensor_copy(sl(2), ssqQ_ps[:])
                chain(sl(0), sl(2))
                bcast(0, 0, 2 * N)
                bcast(1, 0, 2 * N + TL)

                # ---- V phase (PE work overlapping the normalize chains) ----
                wv = wslabs("wv", 2 * C)
                for nh in range(2):
                    for mt in range(8):
                        base = mt * 1040
                        acc = ps.tile([128, 512], F32, tag="ps")
                        for kt in range(KT):
                            nc.tensor.matmul(
                                acc[:],
                                xT[:, kt * N + mt * 128: kt * N + (mt + 1) * 128],
                                wv[nh * KT + kt][:],
                                start=(kt == 0), stop=(kt == KT - 1))
                        dst = vful[:, base + nh * 8 * 65: base + (nh + 1) * 8 * 65]
                        nc.scalar.activation(
                            dst.rearrange("p (h e) -> p h e", h=8)[:, :, 0:64],
                            acc[:].rearrange("p (h e) -> p h e", h=8),
                            AF.Copy)
                for mt in range(8):
                    oc = vful[:, mt * 1040: (mt + 1) * 1040]
                    oc = oc.rearrange("p (h e) -> p h e", h=16)[:, :, 64:65]
                    nc.vector.memset(oc.bitcast(F32), 1.0)

                # ---- normalize K and Q (DVE/ACT, overlaps V matmuls) ----
                with tc.tile_pool(name="ntmp", bufs=3) as ntp:
                    for m in range(8):
                        s = khat[:, m * N:(m + 1) * N]
                        t = ntp.tile([128, N], F32, tag="nt")
                        nc.vector.tensor_mul(t[:], s.bitcast(F32), rstdb[:, 0:N])
                        nc.vector.tensor_sub(t[:], t[:], rstdb[:, N:2 * N])
                        nc.scalar.activation(s, t[:], AF.Identity,
                                             scale=prm(2, m), bias=prm(3, m))
                    for m in range(8):
                        s = qhat[:, m * TL:(m + 1) * TL]
                        t = ntp.tile([128, TL], F32, tag="ntq")
                        nc.vector.tensor_mul(t[:], s.bitcast(F32),
                                             rstdb[:, 2 * N:2 * N + TL])
                        nc.vector.tensor_sub(
                            t[:], t[:], rstdb[:, 2 * N + TL:2 * N + 2 * TL])
                        nc.scalar.activation(s, t[:], AF.Identity,
                                             scale=prm(0, m), bias=prm(1, m))

            # ---------- attention (pairs, software-pipelined depth 2) ----------
            with tc.tile_pool(name="pp", bufs=6) as ppool, \
                 tc.tile_pool(name="att", bufs=2) as attp, \
                 tc.tile_pool(name="osbp", bufs=1) as osbp, \
                 tc.tile_pool(name="wp2", bufs=16) as wpool2:
                osb = osbp.tile([128, KT * TL], F32R, tag="osb")    # 2MB
                dcol = osbp.tile([128, 2048], F32, tag="dcol")
                drec = osbp.tile([128, 2048], F32R, tag="drec")
                wp = [wpool2.tile([128, 512], F32R, tag="w2",
                                  name=f"wp_{i}") for i in range(2 * KT)]
                for grp in range(2):
                    for kt in range(KT):
                        nc.sync.dma_start(
                            wp[grp * KT + kt][:],
                            projwT_d[kt * 128:(kt + 1) * 128,
                                     grp * 512:(grp + 1) * 512])
                qzs = [osbp.tile([128, 1024], F32R, tag=f"qz{i}",
                                 name=f"qz{i}") for i in range(2)]
                for q in qzs:
                    nc.vector.memset(q[:].bitcast(F32), 0.0)

                for kth in range(8):
                    hA, hB = 2 * kth, 2 * kth + 1
                    qz = qzs[kth % 2]
                    nc.vector.tensor_copy(
                        qz[0:64, 0:512], qhat[0:64, kth * TL:(kth + 1) * TL])
                    nc.vector.tensor_copy(
                        qz[64:128, 512:1024],
                        qhat[64:128, kth * TL:(kth + 1) * TL])
                    o_psA = ps.tile([65, 512], F32, tag="ps", name=f"oA{kth}")
                    o_psB = ps.tile([65, 512], F32, tag="ps", name=f"oB{kth}")

                    s_tiles = {}

                    def emit_S(tt, qz=qz, kth=kth, s_tiles=s_tiles):
                        ksl = khat[:, kth * N + tt * 128: kth * N + (tt + 1) * 128]
                        sA = ps.tile([128, 512], F32, tag="ps", name=f"sA{tt}")
                        nc.tensor.matmul(sA[:], ksl, qz[:, 0:512],
                                         start=True, stop=True)
                        sB = ps.tile([128, 512], F32, tag="ps", name=f"sB{tt}")
                        nc.tensor.matmul(sB[:], ksl, qz[:, 512:1024],
                                         start=True, stop=True)
                        s_tiles[tt] = (sA, sB)

                    emit_S(0)
                    emit_S(1)
                    for tt in range(8):
                        sA, sB = s_tiles.pop(tt)
                        pA = ppool.tile([128, 512], F32R, tag="p", name="pA")
                        nc.scalar.activation(pA[:], sA[:], AF.Exp, scale=SCALE)
                        pB = ppool.tile([128, 512], F32R, tag="p", name="pB")
                        nc.scalar.activation(pB[:], sB[:], AF.Exp, scale=SCALE)
                        if tt + 2 < 8:
                            emit_S(tt + 2)
                        nc.tensor.matmul(
                            o_psA[:],
                            vful[:, tt * 1040 + hA * 65: tt * 1040 + (hA + 1) * 65],
                            pA[:], start=(tt == 0), stop=(tt == 7))
                        nc.tensor.matmul(
                            o_psB[:],
                            vful[:, tt * 1040 + hB * 65: tt * 1040 + (hB + 1) * 65],
                            pB[:], start=(tt == 0), stop=(tt == 7))
                    for par, o_ps in ((0, o_psA), (1, o_psB)):
                        s_idx = kth * 2 + par
                        row, col = 32 * (s_idx % 4), 512 * (s_idx // 4)
                        nc.scalar.activation(
                            osb[par * 64:par * 64 + 64, kth * TL:(kth + 1) * TL],
                            o_ps[0:64, :], AF.Copy)
                        nc.vector.tensor_copy(dcol[row:row + 1, col:col + 512],
                                              o_ps[64:65, :])
                # batched reciprocal of all 16 denominators: 1/d = exp(-ln d)
                nc.scalar.activation(dcol[:], dcol[:], AF.Ln)
                nc.scalar.activation(drec[:], dcol[:], AF.Exp, scale=-1.0)

                for kth in range(8):
                    for par in range(2):
                        s_idx = kth * 2 + par
                        row, col = 32 * (s_idx % 4), 512 * (s_idx // 4)
                        bc_ps = ps.tile([128, 512], F32, tag="ps", name="bc")
                        nc.tensor.matmul(bc_ps[:],
                                         ones_big[row:row + 1, :],
                                         drec[row:row + 1, col:col + 512],
                                         start=True, stop=True,
                                         tile_position=(row, 0))
                        rb = attp.tile([128, 512], F32R, tag="rb")
                        nc.vector.tensor_copy(rb[:].bitcast(F32), bc_ps[:])
                        sl_o = osb[par * 64:par * 64 + 64,
                                   kth * TL:(kth + 1) * TL]
                        nc.vector.tensor_mul(
                            sl_o, sl_o.bitcast(F32),
                            rb[par * 64:par * 64 + 64, :].bitcast(F32))
                for wave in range(2):
                    accs = [ps.tile([128, 512], F32, tag="ps",
                                    name=f"pacc{wave}_{m}") for m in range(4)]
                    for kth in range(8):
                        for mi in range(4):
                            m = wave * 4 + mi
                            nc.tensor.matmul(
                                accs[mi][:],
                                wp[(m // 4) * KT + kth][:, (m % 4) * 128:(m % 4 + 1) * 128],
                                osb[:, kth * TL:(kth + 1) * TL],
                                start=(kth == 0), stop=(kth == 7))
                    for mi in range(4):
                        m = wave * 4 + mi
                        ot = attp.tile([128, 512], F32, tag="ot")
                        nc.scalar.activation(ot[:], accs[mi][:], AF.Identity,
                                             bias=prm(6, m))
                        nc.sync.dma_start(out_d[m * 128:(m + 1) * 128, :], ot[:])

    nc.compile()
    return nc


def _get_compiled():
    global _COMPILED
    if _COMPILED is None:
        _COMPILED = _build()
    return _COMPILED


def _host_prep(x, qkv_w, qkv_b, qn_w, qn_b, kn_w, kn_b, proj_w, proj_b):
    qkvwT = np.ascontiguousarray(np.asarray(qkv_w, np.float32).T)
    projwT = np.ascontiguousarray(np.asarray(proj_w, np.float32).T)
    qkv_b = np.asarray(qkv_b, np.float32)
    ws_q = np.asarray(qkv_w, np.float32)[0:C].sum(axis=0)
    ws_k = np.asarray(qkv_w, np.float32)[C:2 * C].sum(axis=0)
    wsums = np.zeros((128, 16), np.float32)
    for kt in range(8):
        wsums[:, kt * 2] = ws_q[kt * 128:(kt + 1) * 128]
        wsums[:, kt * 2 + 1] = ws_k[kt * 128:(kt + 1) * 128]
    bq = qkv_b[0:C].sum() / C
    bk = qkv_b[C:2 * C].sum() / C
    bsumC = np.array([[bq], [bk]], np.float32)
    params = np.zeros((128, 56), np.float32)
    proj_b2 = np.asarray(proj_b, np.float32) + \
        np.asarray(proj_w, np.float32) @ qkv_b[2 * C:3 * C]
    for g, vec in enumerate([qn_w, qn_b, kn_w, kn_b,
                             qkv_b[0:C], qkv_b[C:2 * C], proj_b2]):
        params[:, g * 8:(g + 1) * 8] = \
            np.asarray(vec, np.float32).reshape(8, 128).T
    selc = np.zeros((2, 256), np.float32)
    selc[0, 0:128] = 1.0
    selc[1, 128:256] = 1.0

    in_maps = []
    for c in range(N_CORES):
        b, half = c // 2, c % 2
        xb = np.asarray(x[b], np.float32)
        xr = np.roll(xb, -half * TL, axis=0)   # local tokens -> rows [0,512)
        xT = np.ascontiguousarray(xr.T)
        in_maps.append({
            "xT": xT, "qkvwT": qkvwT, "projwT": projwT, "wsums": wsums,
            "params": params, "bsumC": bsumC, "selc": selc,
        })
    return in_maps


def _run(inputs, trace=False):
    from concourse.bass_utils import run_bass_kernel_spmd
    nc = _get_compiled()
    in_maps = _host_prep(**inputs)
    res = run_bass_kernel_spmd(nc, in_maps, core_ids=list(range(N_CORES)),
                               trace=trace)
    out = np.empty((B, N, C), np.float32)
    for c in range(N_CORES):
        b, half = c // 2, c % 2
        out[b, half * TL:(half + 1) * TL, :] = res.results[c]["out"].T
    return out, res


def kernel(**inputs):
    out, _ = _run(inputs, trace=False)
    return out

